# revision 17
# baseline (speedup 1.0000x reference)
"""Dense dot-product attention (B=16, S=2048, D=128, fp32) on 8 TRN2 NeuronCores.

Sharding: data-parallel over batch — each of the 8 cores processes 2 full
batches independently (no collectives). Measured ~110-112us HW exec
(baseline 126us), rel err ~6.5e-3 vs the 2e-2 gate.

Per-core algorithm (per batch b, D=128, S=2048):

  - Load Q, K naturally ([s, d] fp32), cast to fp16 on DVE, PE-transpose
    into [d, s] QT/KT (fp16 transposes are 1 cycle/row; DMA-XBAR transposes
    were measured at ~1.2us EACH of sync-queue occupancy and elementwise
    engines are 2-2.5x slower than modeled, so the PE does them).
    V is cast to fp16 scaled by CBAR (see below). Batch-0 loads are split
    fine-grained and issue-ordered so each k-group/v-quarter lands just
    before its first consumer (DMA e2e ~2-3.5us); batch b+1's prep runs
    inside batch b's k-loop via deferred steps.
  - Queries in 4 chunks of 512; k-tiles in 8 PAIRS per chunk. Per pair:
      S^T[k, q]  = 2 fp16 matmuls into one [128, 2, 512] PSUM pair tile
      P^T[k, q]  = exp(S^T / sqrt(D)) — ONE ScalarE activation per pair
                   (1024 elem, halves the ~185ns/instr ScalarE bubble);
                   fp16 SBUF out. PE (~1.26us/pair) paces; ScalarE
                   (~1.11us/pair) follows.
      O^T[d, q] += 2 matmuls(lhsT=V fp16, rhs=P^T slot)   (PSUM acc)
      Z[q]      += ONE DoubleRow fp8 matmul (0.5 cyc/row): the HIGH BYTE of
                   an fp16 is exactly its fp8-e5m2 truncation (same exponent
                   bias), so a strided bitcast view of P^T gives free fp8
                   operands; ones[128,2,128] stationary (DoubleRow LDWEIGHTS
                   requires full 128-wide slots) -> [128, 512] PSUM.
                   This makes the softmax-denominator reduction ~4x cheaper
                   than an fp16 ones-matmul and needs no extra cast pass —
                   critically, the elementwise engines (DVE ~2ns/elem, Pool
                   ~2.3ns/elem measured) are 5x too slow to accumulate Z.
                   The truncation's systematic -8.4% bias on Z is exactly
                   compensated by scaling V with CBAR = E[trunc(p)/p]
                   (output = (CBAR S p v) / (CBAR S p) cancels).
  - Chunk end (epilogue deferred into the next chunk's k-loop, split across
    two pair-slots so no engine stalls): copy Z rows 0-8 to SBUF; transpose
    Z columns ([8,128] -> [128,8], only 8 moving rows each since Z is
    replicated); DVE reciprocal; PE-transpose O^T back to [q, d] fp16;
    normalize during PSUM evacuation (tensor_scalar mul by 1/Z[q]); DMA out.

PSUM (8 banks): score pairs 2 tags x 2 banks + outT 1 + z ring 2 + tp 1.
The zt/pst_o transposes share the tp ring in the emission order
zt -> recip -> pst_o -> norm to keep ring dependencies contiguous
(Tile deps are emission-ordered; a read emitted after a later writer on
the same ring slot is a race).
"""

import math
import sys
from contextlib import ExitStack

try:
    import concourse.bass  # noqa: F401
except ImportError:
    for _p in ("/opt/trn_rl_repo", "/root/.axon_site/_ro/trn_rl_repo"):
        if _p not in sys.path:
            sys.path.insert(0, _p)

import numpy as np

import concourse.bass as bass
import concourse.mybir as mybir
import concourse.tile as tile
from concourse import bacc
from concourse.bass_utils import run_bass_kernel_spmd
from concourse.masks import make_identity

B, S, D = 16, 2048, 128
N_CORES = 8
B_LOC = B // N_CORES  # batches per core
P = 128
N_KT = S // P          # k tiles per batch (16)
N_KP = N_KT // 2       # k-tile pairs per chunk (8)
QCHUNK = 512           # queries per accumulation pass
N_QC = S // QCHUNK     # q chunks per batch (4)
NQT = QCHUNK // P      # output q tiles per chunk (4)
SOFTMAX_SCALE = 1.0 / math.sqrt(D)
# mean of trunc_e5m2(p)/p over the exp(N(0,1)) score distribution; folded
# into the V cast so the Z truncation bias cancels exactly in the softmax
CBAR = 0.91573

F32 = mybir.dt.float32
F16 = mybir.dt.float16
F8E5 = mybir.dt.float8e5


def build_attention_nc() -> bass.Bass:
    nc = bacc.Bacc()
    q_in = nc.declare_dram_parameter("query", [B_LOC, S, D], F32, isOutput=False)
    k_in = nc.declare_dram_parameter("key", [B_LOC, S, D], F32, isOutput=False)
    v_in = nc.declare_dram_parameter("value", [B_LOC, S, D], F32, isOutput=False)
    o_out = nc.declare_dram_parameter("out", [B_LOC, S, D], F32, isOutput=True)

    with tile.TileContext(nc) as tc, ExitStack() as ctx:
        const = ctx.enter_context(tc.tile_pool(name="const", bufs=1))
        io = ctx.enter_context(tc.tile_pool(name="io", bufs=2))
        tr = ctx.enter_context(tc.tile_pool(name="tr", bufs=2))
        pexp = ctx.enter_context(tc.tile_pool(name="pexp", bufs=4))
        norm = ctx.enter_context(tc.tile_pool(name="norm", bufs=2))
        # PSUM: sc pairs 2x2 banks + outT 1 + z ring 2 + tp 1 = 8 banks
        ps_s = ctx.enter_context(tc.tile_pool(name="ps_s", bufs=2, space="PSUM"))
        ps_acc = ctx.enter_context(tc.tile_pool(name="ps_acc", bufs=1, space="PSUM"))

        identity = const.tile([P, P], F32)
        make_identity(nc, identity)
        identity_h = const.tile([P, P], F16)
        nc.vector.tensor_copy(identity_h[:], identity[:])
        # DoubleRow LDWEIGHTS requires full 128-wide stationary slots
        # (s3_lw_dual_fp8_restrictions rejects narrow ones)
        ones2x128_f = const.tile([P, 2, P], F32)
        nc.gpsimd.memset(ones2x128_f[:], 1.0)
        ones2x128_e5 = const.tile([P, 2, P], F8E5)
        nc.vector.tensor_copy(ones2x128_e5[:], ones2x128_f[:])

        pending_z = None
        pending_fin = None

        # ---- per-batch input prep, split into pipelinable steps ----
        def emit_v_part(v_nat, v_f16, b, part, nparts):
            ntile = N_KT // nparts
            sl = slice(part * ntile, (part + 1) * ntile)
            nc.sync.dma_start(
                v_nat[:, sl, :],
                v_in[b, part * ntile * P : (part + 1) * ntile * P, :].rearrange(
                    "(t p) d -> p t d", p=P
                ),
            )
            # CBAR compensates the e5m2-truncated softmax denominator
            nc.vector.tensor_scalar_mul(v_f16[:, sl, :], v_nat[:, sl, :], CBAR)

        def emit_qk_load(src_in, b, j4, tagp, split=False):
            nat = io.tile(
                [P, 4, D], F32, tag="qknat", name=f"nat_{tagp}_{b}_{j4}", bufs=8
            )
            rnd = io.tile(
                [P, 4, D], F16, tag="qkrnd", name=f"rnd_{tagp}_{b}_{j4}", bufs=8
            )
            # split=True: halve the first DMA + cast so downstream PE
            # transposes start ~1.5us earlier at kernel startup
            parts = ((0, 2), (2, 4)) if split else ((0, 4),)
            for lo, hi in parts:
                nc.sync.dma_start(
                    nat[:, lo:hi, :],
                    src_in[b, (j4 * 4 + lo) * P : (j4 * 4 + hi) * P, :].rearrange(
                        "(t p) d -> p t d", p=P
                    ),
                )
                nc.vector.tensor_copy(rnd[:, lo:hi, :], nat[:, lo:hi, :])
            return rnd

        def emit_qk_transp(rnd, b, j4, dst):
            pst = ps_s.tile([P, 4, P], F16, tag="tp", name=f"pst_{b}_{j4}", bufs=1)
            for jj in range(4):
                nc.tensor.transpose(pst[:, jj, :], rnd[:, jj, :], identity_h[:])
            nc.vector.tensor_copy(
                dst[:, j4 * 4 * P : (j4 + 1) * 4 * P], pst[:]
            )

        def make_prep_steps(b):
            """Returns (qt, kt, v_f16, steps, deferred)."""
            qt = tr.tile([P, S], F16, tag="qt", name=f"qt_{b}")
            kt = tr.tile([P, S], F16, tag="kt", name=f"kt_{b}")
            v_nat = io.tile([P, N_KT, D], F32, tag="vnat", name=f"vnat_{b}")
            v_f16 = io.tile([P, N_KT, D], F16, tag="vf16", name=f"vf16_{b}")

            def qk_split(src_in, j4, dst, tagp, split=False):
                box = {}

                def load():
                    box["rnd"] = emit_qk_load(src_in, b, j4, tagp, split=split)

                def transp():
                    emit_qk_transp(box["rnd"], b, j4, dst)

                return load, transp

            kp_ = [
                qk_split(k_in, j4, kt, "k", split=(b == 0))
                for j4 in range(N_KT // 4)
            ]
            qp_ = [
                qk_split(q_in, j4, qt, "q", split=(b == 0 and j4 == 0))
                for j4 in range(N_KT // 4)
            ]

            if b == 0:
                # issue order tuned so every k group and v quarter lands just
                # before its first consumer in chunk 0 (DMA e2e ~2-3.5us)
                steps = [
                    kp_[0][0], qp_[0][0],
                    kp_[0][1], qp_[0][1],
                    kp_[1][0],
                    lambda: emit_v_part(v_nat, v_f16, b, 0, 4),
                    kp_[2][0],
                    lambda: emit_v_part(v_nat, v_f16, b, 1, 4),
                    kp_[3][0],
                    lambda: emit_v_part(v_nat, v_f16, b, 2, 4),
                    lambda: emit_v_part(v_nat, v_f16, b, 3, 4),
                    qp_[1][0], qp_[2][0], qp_[3][0],
                ]
                deferred = [
                    kp_[1][1], kp_[2][1], kp_[3][1],
                    qp_[1][1], qp_[2][1], qp_[3][1],
                ]
            else:
                steps = []
                deferred = [
                    kp_[0][0], kp_[1][0], kp_[2][0], kp_[3][0],
                    qp_[0][0],
                    lambda: emit_v_part(v_nat, v_f16, b, 0, 2),
                    lambda: emit_v_part(v_nat, v_f16, b, 1, 2),
                    kp_[0][1], kp_[1][1], kp_[2][1], kp_[3][1],
                    qp_[0][1],
                    qp_[1][0], qp_[2][0], qp_[3][0],
                    qp_[1][1], qp_[2][1], qp_[3][1],
                ]
            return qt, kt, v_f16, steps, deferred

        prep = {0: make_prep_steps(0)}
        deferred_steps: list = []

        for b in range(B_LOC):
            qt, kt, v_f16, steps, deferred = prep[b]
            for st in steps:
                st()
            deferred_steps.extend(deferred)
            steps.clear()

            if b + 1 < B_LOC:
                prep[b + 1] = make_prep_steps(b + 1)
                deferred_steps.extend(prep[b + 1][3])
                deferred_steps.extend(prep[b + 1][4])
                prep[b + 1][3].clear()
                prep[b + 1][4].clear()

            def emit_sc_pair(q_lo, kp):
                sc = ps_s.tile(
                    [P, 2, QCHUNK], F32, tag=f"sc{kp % 2}", name=f"sc_{kp}", bufs=1
                )
                for i in range(2):
                    nc.tensor.matmul(
                        sc[:, i, :],
                        kt[:, (2 * kp + i) * P : (2 * kp + i + 1) * P],
                        qt[:, q_lo : q_lo + QCHUNK],
                        start=True,
                        stop=True,
                    )
                return sc

            def emit_epilogue_z(z_ps, z_sb):
                # Z replicated across partitions; only 8 rows needed for the
                # cheap [8,128] -> [128,8] transposes
                nc.vector.tensor_copy(z_sb[:], z_ps[:8, :])

            def emit_epilogue_fin(b, q_lo, o_un, z_sb):
                # transpose Z columns: [8, 128] -> [128, 8] (8 moving rows);
                # zt/pst_o share the tp ring, so emission order must be
                # zt -> recip -> pst_o -> norm to keep ring deps contiguous
                zt_ps = ps_s.tile([P, NQT, 8], F32, tag="tp", name="zt_ps", bufs=1)
                for j in range(NQT):
                    nc.tensor.transpose(
                        zt_ps[:, j, :],
                        z_sb[:8, j * P : (j + 1) * P],
                        identity[:8, :8],
                    )
                zr = norm.tile([P, NQT], F32, tag="zr")
                nc.vector.reciprocal(zr[:], zt_ps[:, :, 0])

                pst = ps_s.tile([P, NQT, D], F16, tag="tp", name="pst_o", bufs=1)
                for j in range(NQT):
                    nc.tensor.transpose(
                        pst[:, j, :], o_un[:, j * P : (j + 1) * P], identity_h[:]
                    )
                out_sb = norm.tile([P, NQT, D], F32, tag="osb")
                for j in range(NQT):
                    nc.vector.tensor_scalar_mul(
                        out_sb[:, j, :], pst[:, j, :], zr[:, j : j + 1]
                    )
                nc.sync.dma_start(
                    o_out[b, q_lo : q_lo + QCHUNK, :].rearrange(
                        "(t p) d -> p t d", p=P
                    ),
                    out_sb[:],
                )

            for qc in range(N_QC):
                q_lo = qc * QCHUNK
                outT = ps_acc.tile([P, QCHUNK], F32, tag="outT", name="outT")
                z_ps = ps_s.tile([P, QCHUNK], F32, tag="z", name="z_ps", bufs=2)

                sc = emit_sc_pair(q_lo, 0)
                for kp in range(N_KP):
                    pt = pexp.tile(
                        [P, 2, QCHUNK], F16, tag="pt", name=f"pt_{kp}", bufs=4
                    )
                    nc.scalar.activation(
                        pt[:],
                        sc[:],
                        mybir.ActivationFunctionType.Exp,
                        scale=SOFTMAX_SCALE,
                    )
                    # deferred prep must be emitted BEFORE the next score
                    # pair: Tile deps are emission-ordered, and sc(kp+1) may
                    # read kt columns written by a deferred transpose
                    if kp == 1 and pending_z is not None:
                        pending_z()
                        pending_z = None
                    elif kp == 2 and pending_fin is not None:
                        pending_fin()
                        pending_fin = None
                    elif deferred_steps and kp >= 1:
                        deferred_steps.pop(0)()
                    if kp + 1 < N_KP:
                        sc = emit_sc_pair(q_lo, kp + 1)
                    for i in range(2):
                        nc.tensor.matmul(
                            outT[:],
                            v_f16[:, 2 * kp + i, :],
                            pt[:, i, :],
                            start=(kp == 0 and i == 0),
                            stop=(kp == N_KP - 1 and i == 1),
                        )
                    # softmax denominator: DoubleRow fp8 matmul on the
                    # high-byte (e5m2) view of the fp16 P pair
                    pt8 = pt.bitcast(F8E5)
                    nc.tensor.matmul(
                        z_ps[:],
                        ones2x128_e5[:],
                        pt8[:, :, 1::2],
                        start=(kp == 0),
                        stop=(kp == N_KP - 1),
                        perf_mode=mybir.MatmulPerfMode.DoubleRow,
                    )

                # evacuate accumulator (frees the PSUM bank for next chunk)
                o_un = norm.tile([P, QCHUNK], F16, tag="o_un")
                nc.vector.tensor_copy(o_un[:], outT[:])
                z_sb = norm.tile([8, QCHUNK], F32, tag="z_sb")

                pending_z = (
                    lambda z_ps=z_ps, z_sb=z_sb: emit_epilogue_z(z_ps, z_sb)
                )
                pending_fin = (
                    lambda b=b, q_lo=q_lo, o_un=o_un, z_sb=z_sb: emit_epilogue_fin(
                        b, q_lo, o_un, z_sb
                    )
                )

        if pending_z is not None:
            pending_z()
        if pending_fin is not None:
            pending_fin()
        while deferred_steps:
            deferred_steps.pop(0)()

    nc.compile()
    return nc


_NC_CACHE: bass.Bass | None = None


def _get_nc() -> bass.Bass:
    global _NC_CACHE
    if _NC_CACHE is None:
        _NC_CACHE = build_attention_nc()
    return _NC_CACHE


def kernel(query: np.ndarray, key: np.ndarray, value: np.ndarray) -> np.ndarray:
    query = np.ascontiguousarray(np.asarray(query, dtype=np.float32))
    key = np.ascontiguousarray(np.asarray(key, dtype=np.float32))
    value = np.ascontiguousarray(np.asarray(value, dtype=np.float32))
    assert query.shape == (B, S, D), query.shape

    nc = _get_nc()
    core_ids = list(range(N_CORES))
    in_maps = [
        {
            "query": query[i * B_LOC : (i + 1) * B_LOC],
            "key": key[i * B_LOC : (i + 1) * B_LOC],
            "value": value[i * B_LOC : (i + 1) * B_LOC],
        }
        for i in range(N_CORES)
    ]
    res = run_bass_kernel_spmd(nc, in_maps, core_ids)
    out = np.concatenate([res.results[i]["out"] for i in range(N_CORES)], axis=0)
    return out


if __name__ == "__main__":
    rng = np.random.default_rng(0)
    q = rng.standard_normal((B, S, D)).astype(np.float32)
    k = rng.standard_normal((B, S, D)).astype(np.float32)
    v = rng.standard_normal((B, S, D)).astype(np.float32)
    o = kernel(q, k, v)
    print("out", o.shape, o.dtype, float(np.abs(o).max()))
